# revision 2
# baseline (speedup 1.0000x reference)
"""MFDWC feature extractor, v8b: v8 + fp16 host waveform (no cast stage).

Differences vs v1:
  - Pre-emphasis is folded into the DFT chunk matrices (contraction runs
    over 442-sample raw chunks; x[-1]=0 pad gives emph[0]=x[0] exactly),
    so phase 1 is just cast fp32->fp8 + PE transpose.
  - DFT and mel matmuls run in fp8e4 with perf_mode=DoubleRow (2 k-subtiles
    per instruction): weights [128, 2, M] / moving [128, 2, N] APs.
  - Power is computed as (X/64)^2 in fp8 (range fits e4m3); the 4096x is
    restored inside the log: Ln(4096*mel' + eps) == Ln(mel + eps) exactly.
  - Phases ordered A(r0) A(r1) B(r0) B(r1) C(r0) C(r1) so the per-row
    stats tails don't head-of-line-block the other row's pipeline.
"""

import math
from contextlib import ExitStack

import numpy as np
import ml_dtypes

import concourse.bass as bass
import concourse.bacc as bacc
import concourse.mybir as mybir
import concourse.tile as tile
from concourse.bass_utils import run_bass_kernel_spmd

F32 = mybir.dt.float32
F16 = mybir.dt.float16
F8 = mybir.dt.float8e4
AF = mybir.ActivationFunctionType
DR = mybir.MatmulPerfMode.DoubleRow
F8NP = ml_dtypes.float8_e4m3

B = 16               # batch
L = 441000           # samples per row
W = 441              # hop; chunks of 442 rows (x[441t-1 .. 441t+440])
NK = 1000            # chunk columns per row
T = 999              # frames per row
NB = 1024            # matmul bins (Nyquist packed into sin col 0)
NMEL = 60
ROWS = 2             # batch rows per core
EPS = 1e-10
SQRT2 = math.sqrt(2.0)
XSC = 1.0 / 64.0     # fold |X|/64 into the squares; 4096x restored in Ln

# chunk column tiling (8 DMA chunks over the 1000 columns)
ECH = [(k * 128, min(128, NK - k * 128)) for k in range(8)]
# transpose row-blocks over the 442 rows per chunk
RBL = [(0, 128), (128, 128), (256, 128), (384, 58)]
# frame chunks (PSUM free-dim <= 512 fp32)
FCH = [(0, 512), (512, 487)]


def _host_constants(mel_filters: np.ndarray):
    j = np.arange(882, dtype=np.float64)
    b = np.arange(NB, dtype=np.float64)
    ham = np.hamming(882)
    ang = 2.0 * np.pi * np.outer(j, b) / 2048.0
    cw = ham[:, None] * np.cos(ang)                               # (882, 1024)
    sw = ham[:, None] * np.sin(ang)
    sw[:, 0] = ham * np.cos(np.pi * j)                            # Nyquist col

    def fold(M):
        # chunk-matrix pair with pre-emphasis folded in (see fold_check.py)
        G0 = np.zeros((512, NB))
        G1 = np.zeros((512, NB))
        G0[1:442, :] = M[0:441, :]
        G0[0:442, :] -= 0.97 * M[0:442, :]
        G1[1:441, :] = M[441:881, :] - 0.97 * M[442:882, :]
        G1[441, :] = M[881, :]
        # [512, 1024] -> [128, 4*1024] (row p holds matrix rows 128g+p at g*1024+b)
        return np.ascontiguousarray(
            G0.reshape(4, 128, NB).transpose(1, 0, 2)
        ).astype(F8NP), np.ascontiguousarray(
            G1.reshape(4, 128, NB).transpose(1, 0, 2)
        ).astype(F8NP)

    g0c, g1c = fold(cw)
    g0s, g1s = fold(sw)

    m = mel_filters.astype(np.float64)                            # (60, 1025)
    mat = m[:, 0:NB]                                              # for |cos| part
    mbt = np.concatenate([m[:, NB:NB + 1], m[:, 1:NB]], axis=1)   # Nyquist in col 0
    # melW[p, 2*bc + {0,1}, m] = {mat,mbt}[m, bc*128 + p], padded m->64
    melw = np.zeros((128, 16, 64), np.float64)
    for bc in range(8):
        melw[:, 2 * bc + 0, 0:NMEL] = mat[:, bc * 128:(bc + 1) * 128].T
        melw[:, 2 * bc + 1, 0:NMEL] = mbt[:, bc * 128:(bc + 1) * 128].T
    melw = np.ascontiguousarray(melw).astype(F8NP)

    idn = np.eye(128, dtype=np.float16)
    hsum = np.zeros((NMEL, 30), np.float16)
    hdif = np.zeros((NMEL, 30), np.float16)
    for i in range(30):
        hsum[2 * i, i] = 1.0
        hsum[2 * i + 1, i] = 1.0
        hdif[2 * i, i] = 1.0
        hdif[2 * i + 1, i] = -1.0
    return g0c, g1c, g0s, g1s, melw, idn, hsum, hdif


def _body(ctx: ExitStack, tc, xpad, g0c_d, g1c_d, g0s_d, g1s_d, melw_d, idn_d,
          hs_d, hd_d, out_d):
    nc = tc.nc

    const = ctx.enter_context(tc.tile_pool(name="const", bufs=1))
    e2p = ctx.enter_context(tc.tile_pool(name="e2", bufs=3))
    etp = ctx.enter_context(tc.tile_pool(name="et", bufs=1))
    ptrp = ctx.enter_context(tc.tile_pool(name="ptr", bufs=1, space="PSUM"))
    dftp = ctx.enter_context(tc.tile_pool(name="dft", bufs=3, space="PSUM"))
    melp = ctx.enter_context(tc.tile_pool(name="mel", bufs=1, space="PSUM"))
    ppp = ctx.enter_context(tc.tile_pool(name="pp", bufs=4))
    lmp = ctx.enter_context(tc.tile_pool(name="lm", bufs=1))
    hop = ctx.enter_context(tc.tile_pool(name="ho", bufs=1))
    stp = ctx.enter_context(tc.tile_pool(name="st", bufs=2))

    # ---- constants
    wt = {}
    for nm, d in (("g0c", g0c_d), ("g1c", g1c_d), ("g0s", g0s_d), ("g1s", g1s_d)):
        t = const.tile([128, 4, NB], F8, tag=nm, name=nm)
        nc.gpsimd.dma_start(t[:, :, :], d[:, :, :])
        wt[nm] = t
    melw = const.tile([128, 16, 64], F8, tag="melw", name="melw")
    nc.gpsimd.dma_start(melw[:, :, :], melw_d[:, :, :])
    ident = const.tile([128, 128], F16, tag="id", name="ident")
    nc.gpsimd.dma_start(ident[:, :], idn_d[:, :])
    eps_t = const.tile([128, 1], F32, tag="eps", name="eps")
    nc.vector.memset(eps_t[:, :], EPS)
    hs_t = const.tile([NMEL, 30], F16, tag="hs", name="hs")
    nc.gpsimd.dma_start(hs_t[:, :], hs_d[:, :])
    hd_t = const.tile([NMEL, 30], F16, tag="hd", name="hd")
    nc.gpsimd.dma_start(hd_t[:, :], hd_d[:, :])

    # ---- phase A: load + cast fp8 + transpose to et[r] = [128, 4, 1024]
    et = {}

    def phase_a_open(r):
        etr = etp.tile([128, 4, NK + 24], F8, tag=f"et{r}", name=f"et{r}")
        nc.vector.memset(etr[:, 3, :], 0.0)   # pad rows 442..511 (0..57 rewritten)
        et[r] = etr

    def phase_a_chunks(r, c_lo, c_hi):
        etr = et[r]
        for ci in range(c_lo, c_hi):
            k0, ksz = ECH[ci]
            e2 = e2p.tile([128, W + 1], F16, tag="e2", name="e2")
            src = bass.AP(xpad, r * (L + 1) + W * k0, [[W, ksz], [1, W + 1]])
            nc.sync.dma_start(e2[0:ksz, :], src)
            ptr4 = ptrp.tile([128, 4, 128], F16, tag="ptr", name="ptr4")
            for rb, (rb0, rbsz) in enumerate(RBL):
                nc.tensor.matmul(ptr4[0:rbsz, rb, 0:ksz], e2[0:ksz, rb0:rb0 + rbsz],
                                 ident[0:ksz, 0:ksz], is_transpose=True,
                                 skip_group_check=True)
            nc.vector.tensor_copy(etr[:, 0:3, k0:k0 + ksz], ptr4[:, 0:3, 0:ksz])
            nc.vector.tensor_copy(etr[0:58, 3, k0:k0 + ksz], ptr4[0:58, 3, 0:ksz])

    # ---- phase B: DFT power -> mel -> log (all fp8 DoubleRow)
    lmt = {}

    def phase_b_fc(r, fci):
        if r not in lmt:
            lmt[r] = lmp.tile([NMEL, T], F16, tag=f"lm{r}", name=f"lm{r}")
        lmr = lmt[r]
        for (f0, fN) in [FCH[fci]]:
            mp = melp.tile([NMEL, 512], F32, tag="mp", name="mp")
            pend = None                     # delayed mel matmul (ACT slack)
            for bc in range(8):
                pq = dftp.tile([128, 2, 512], F32, tag="pq", name="pq")
                for half, w0, w1 in ((0, wt["g0c"], wt["g1c"]),
                                     (1, wt["g0s"], wt["g1s"])):
                    for mi, (wgt, a, u) in enumerate(
                            ((w0, 0, 0), (w0, 0, 1), (w1, 1, 0), (w1, 1, 1))):
                        nc.tensor.matmul(
                            pq[:, half, 0:fN],
                            wgt[:, 2 * u:2 * u + 2, bc * 128:(bc + 1) * 128],
                            et[r][:, 2 * u:2 * u + 2, f0 + a:f0 + a + fN],
                            start=(mi == 0), stop=(mi == 3), perf_mode=DR,
                            skip_group_check=True)
                    if pend is not None:    # one-bc-delayed mel accumulation
                        nc.tensor.matmul(*pend[0], **pend[1])
                        pend = None
                pp = ppp.tile([128, 2, 512], F8, tag="pp", name="pp")
                nc.scalar.activation(pp[:, :, 0:fN], pq[:, :, 0:fN], AF.Square,
                                     scale=XSC)
                pend = ((mp[0:NMEL, 0:fN],
                         melw[:, 2 * bc:2 * bc + 2, 0:NMEL],
                         pp[:, :, 0:fN]),
                        dict(start=(bc == 0), stop=(bc == 7), perf_mode=DR,
                             skip_group_check=True))
            nc.tensor.matmul(*pend[0], **pend[1])
            nc.scalar.activation(lmr[0:NMEL, f0:f0 + fN], mp[0:NMEL, 0:fN],
                                 AF.Ln, bias=eps_t[0:NMEL, :], scale=1.0 / (XSC * XSC))
        return lmr

    # ---- phase C: Haar / delta / stats
    def phase_c_haar(r, lmr):
        ca = hop.tile([30, T], F32, tag=f"ca{r}", name=f"ca{r}")
        cd = hop.tile([30, T], F32, tag=f"cd{r}", name=f"cd{r}")
        for (f0, fN) in FCH:
            pca = melp.tile([30, 512], F32, tag="mp", name="pca")
            nc.tensor.matmul(pca[:, 0:fN], hs_t[:, :], lmr[0:NMEL, f0:f0 + fN],
                             start=True, stop=True, skip_group_check=True)
            nc.vector.tensor_copy(ca[:, f0:f0 + fN], pca[:, 0:fN])
            pcd = melp.tile([30, 512], F32, tag="mp", name="pcd")
            nc.tensor.matmul(pcd[:, 0:fN], hd_t[:, :], lmr[0:NMEL, f0:f0 + fN],
                             start=True, stop=True, skip_group_check=True)
            nc.vector.tensor_copy(cd[:, f0:f0 + fN], pcd[:, 0:fN])
        dl = hop.tile([30, T], F32, tag=f"dl{r}", name=f"dl{r}")
        nc.vector.tensor_sub(dl[:, 1:T - 1], ca[:, 2:T], ca[:, 0:T - 2])
        nc.vector.tensor_sub(dl[:, 0:1], ca[:, 1:2], ca[:, 0:1])
        nc.vector.tensor_sub(dl[:, T - 1:T], ca[:, T - 1:T], ca[:, T - 2:T - 1])
        return ca, dl, cd

    sqs, statss = {}, {}

    def phase_c_scr(r, ca, dl, cd):
        stats = stp.tile([30, 6], F32, tag=f"stats{r}", name=f"stats{r}")
        sq = stp.tile([30, 3], F32, tag=f"sq{r}", name=f"sq{r}")
        statss[r] = stats
        sqs[r] = sq
        for si, feat in enumerate((ca, dl, cd)):
            s1 = stp.tile([30, 1], F32, tag="s1", name="s1")
            nc.vector.tensor_reduce(s1[:, :], feat[:, :],
                                    axis=mybir.AxisListType.X,
                                    op=mybir.AluOpType.add)
            nc.vector.tensor_scalar_mul(stats[:, si:si + 1], s1[:, :],
                                        1.0 / (T * SQRT2))
            nm = stp.tile([30, 1], F32, tag="nm", name="nm")
            nc.vector.tensor_scalar_mul(nm[:, :], s1[:, :], -1.0 / T)
            scr = stp.tile([30, T], F32, tag="scr", name="scr")
            nc.scalar.activation(scr[:, :], feat[:, :], AF.Square,
                                 bias=nm[:, :], scale=1.0,
                                 accum_out=sq[:, si:si + 1])

    def phase_c_sqrt_out():
        for r in range(ROWS):
            nc.scalar.activation(statss[r][:, 3:6], sqs[r][:, :], AF.Sqrt,
                                 scale=1.0 / ((T - 1) * 2.0))
            nc.sync.dma_start(bass.AP(out_d, r * 180, [[1, 180]]),
                              statss[r][:, :])

    phase_a_open(0)
    phase_a_chunks(0, 0, 5)      # fc0 reads cols 0..512 -> chunks 0..4
    phase_b_fc(0, 0)
    phase_a_chunks(0, 5, 8)
    phase_b_fc(0, 1)
    lm0 = lmt[0]
    f0 = phase_c_haar(0, lm0)
    phase_a_open(1)
    phase_a_chunks(1, 0, 5)
    phase_b_fc(1, 0)
    phase_a_chunks(1, 5, 8)
    phase_b_fc(1, 1)
    lm1 = lmt[1]
    phase_c_scr(0, *f0)
    f1 = phase_c_haar(1, lm1)
    phase_c_scr(1, *f1)
    phase_c_sqrt_out()


_CACHE = {}


def _build():
    if "nc" in _CACHE:
        return _CACHE["nc"]
    nc = bacc.Bacc("TRN2", target_bir_lowering=False, debug=False,
                   enable_asserts=False, num_devices=8)
    xpad = nc.dram_tensor("xpad", [ROWS, L + 1], F16, kind="ExternalInput")
    g0c_d = nc.dram_tensor("g0c", [128, 4, NB], F8, kind="ExternalInput")
    g1c_d = nc.dram_tensor("g1c", [128, 4, NB], F8, kind="ExternalInput")
    g0s_d = nc.dram_tensor("g0s", [128, 4, NB], F8, kind="ExternalInput")
    g1s_d = nc.dram_tensor("g1s", [128, 4, NB], F8, kind="ExternalInput")
    melw_d = nc.dram_tensor("melw", [128, 16, 64], F8, kind="ExternalInput")
    idn_d = nc.dram_tensor("idn", [128, 128], F16, kind="ExternalInput")
    hs_d = nc.dram_tensor("hsum", [NMEL, 30], F16, kind="ExternalInput")
    hd_d = nc.dram_tensor("hdif", [NMEL, 30], F16, kind="ExternalInput")
    out_d = nc.dram_tensor("out", [ROWS, 180], F32, kind="ExternalOutput")
    with tile.TileContext(nc) as tc, ExitStack() as ctx:
        _body(ctx, tc, xpad, g0c_d, g1c_d, g0s_d, g1s_d, melw_d, idn_d,
              hs_d, hd_d, out_d)
    nc.compile()
    _CACHE["nc"] = nc
    return nc


def make_in_maps(waveform: np.ndarray, mel_filters: np.ndarray):
    g0c, g1c, g0s, g1s, melw, idn, hsum, hdif = _host_constants(mel_filters)
    in_maps = []
    for core in range(8):
        rows = waveform[ROWS * core:ROWS * (core + 1)]
        xpad = np.zeros((ROWS, L + 1), np.float16)
        xpad[:, 1:] = rows.astype(np.float16)
        in_maps.append({"xpad": xpad, "g0c": g0c, "g1c": g1c, "g0s": g0s,
                        "g1s": g1s, "melw": melw, "idn": idn,
                        "hsum": hsum, "hdif": hdif})
    return in_maps


def gather_out(results):
    full = np.concatenate([results[c]["out"] for c in range(8)], axis=0)
    return np.ascontiguousarray(
        full.reshape(B, 30, 6).transpose(0, 2, 1).reshape(B, 180)).astype(np.float32)


def run(waveform, mel_filters, trace=False):
    nc = _build()
    in_maps = make_in_maps(np.asarray(waveform, np.float32),
                           np.asarray(mel_filters, np.float32))
    res = run_bass_kernel_spmd(nc, in_maps, core_ids=list(range(8)), trace=trace)
    return gather_out(res.results), res


def kernel(waveform: np.ndarray, mel_filters: np.ndarray) -> np.ndarray:
    out, _ = run(waveform, mel_filters, trace=False)
    return out


# revision 3
# speedup vs baseline: 1.1022x; 1.1022x over previous
"""MFDWC feature extractor, v14: v13 + halved weight DMAs.

Differences vs v1:
  - Pre-emphasis is folded into the DFT chunk matrices (contraction runs
    over 442-sample raw chunks; x[-1]=0 pad gives emph[0]=x[0] exactly),
    so phase 1 is just cast fp32->fp8 + PE transpose.
  - DFT and mel matmuls run in fp8e4 with perf_mode=DoubleRow (2 k-subtiles
    per instruction): weights [128, 2, M] / moving [128, 2, N] APs.
  - Power is computed as (X/64)^2 in fp8 (range fits e4m3); the 4096x is
    restored inside the log: Ln(4096*mel' + eps) == Ln(mel + eps) exactly.
  - Phases ordered A(r0) A(r1) B(r0) B(r1) C(r0) C(r1) so the per-row
    stats tails don't head-of-line-block the other row's pipeline.
"""

import math
from contextlib import ExitStack

import numpy as np
import ml_dtypes

import concourse.bass as bass
import concourse.bacc as bacc
import concourse.mybir as mybir
import concourse.tile as tile
from concourse.bass_utils import run_bass_kernel_spmd

F32 = mybir.dt.float32
F16 = mybir.dt.float16
F8 = mybir.dt.float8e4
AF = mybir.ActivationFunctionType
DR = mybir.MatmulPerfMode.DoubleRow
F8NP = ml_dtypes.float8_e4m3

B = 16               # batch
L = 441000           # samples per row
W = 441              # hop; chunks of 442 rows (x[441t-1 .. 441t+440])
NK = 1000            # chunk columns per row
T = 999              # frames per row
NB = 1024            # matmul bins (Nyquist packed into sin col 0)
NMEL = 60
ROWS = 2             # batch rows per core
EPS = 1e-10
SQRT2 = math.sqrt(2.0)
XSC = 1.0 / 64.0     # fold |X|/64 into the squares; 4096x restored in Ln

# chunk column tiling (8 DMA chunks over the 1000 columns)
ECH = [(k * 128, min(128, NK - k * 128)) for k in range(8)]
# transpose row-blocks over the 442 rows per chunk
RBL = [(0, 128), (128, 128), (256, 128), (384, 58)]
# frame chunks (PSUM free-dim <= 512 fp32)
FCH = [(0, 512), (512, 487)]


def _host_constants(mel_filters: np.ndarray):
    j = np.arange(882, dtype=np.float64)
    b = np.arange(NB, dtype=np.float64)
    ham = np.hamming(882)
    ang = 2.0 * np.pi * np.outer(j, b) / 2048.0
    cw = ham[:, None] * np.cos(ang)                               # (882, 1024)
    sw = ham[:, None] * np.sin(ang)
    sw[:, 0] = ham * np.cos(np.pi * j)                            # Nyquist col

    def fold(M):
        # chunk-matrix pair with pre-emphasis folded in (see fold_check.py)
        G0 = np.zeros((512, NB))
        G1 = np.zeros((512, NB))
        G0[1:442, :] = M[0:441, :]
        G0[0:442, :] -= 0.97 * M[0:442, :]
        G1[1:441, :] = M[441:881, :] - 0.97 * M[442:882, :]
        G1[441, :] = M[881, :]
        # [512, 1024] -> [128, 4*1024] (row p holds matrix rows 128g+p at g*1024+b)
        return np.ascontiguousarray(
            G0.reshape(4, 128, NB).transpose(1, 0, 2)
        ).astype(F8NP), np.ascontiguousarray(
            G1.reshape(4, 128, NB).transpose(1, 0, 2)
        ).astype(F8NP)

    g0c, g1c = fold(cw)
    g0s, g1s = fold(sw)

    m = mel_filters.astype(np.float64)                            # (60, 1025)
    mat = m[:, 0:NB]                                              # for |cos| part
    mbt = np.concatenate([m[:, NB:NB + 1], m[:, 1:NB]], axis=1)   # Nyquist in col 0
    # melW[p, 2*bc + {0,1}, m] = {mat,mbt}[m, bc*128 + p], padded m->64
    melw = np.zeros((128, 16, 64), np.float64)
    for bc in range(8):
        melw[:, 2 * bc + 0, 0:NMEL] = mat[:, bc * 128:(bc + 1) * 128].T
        melw[:, 2 * bc + 1, 0:NMEL] = mbt[:, bc * 128:(bc + 1) * 128].T
    melw = np.ascontiguousarray(melw).astype(F8NP)

    idn = np.eye(128, dtype=np.float16)
    hsum = np.zeros((NMEL, 30), np.float16)
    hdif = np.zeros((NMEL, 30), np.float16)
    for i in range(30):
        hsum[2 * i, i] = 1.0
        hsum[2 * i + 1, i] = 1.0
        hdif[2 * i, i] = 1.0
        hdif[2 * i + 1, i] = -1.0
    return g0c, g1c, g0s, g1s, melw, idn, hsum, hdif


def _body(ctx: ExitStack, tc, xpad, g0c_d, g1c_d, g0s_d, g1s_d, melw_d, idn_d,
          hs_d, hd_d, out_d):
    nc = tc.nc

    const = ctx.enter_context(tc.tile_pool(name="const", bufs=1))
    e2p = ctx.enter_context(tc.tile_pool(name="e2", bufs=8))
    etp = ctx.enter_context(tc.tile_pool(name="et", bufs=1))
    ptrp = ctx.enter_context(tc.tile_pool(name="ptr", bufs=1, space="PSUM"))
    dftp = ctx.enter_context(tc.tile_pool(name="dft", bufs=2, space="PSUM"))
    haarp = ctx.enter_context(tc.tile_pool(name="haar", bufs=1, space="PSUM"))
    melp = ctx.enter_context(tc.tile_pool(name="mel", bufs=1, space="PSUM"))
    ppp = ctx.enter_context(tc.tile_pool(name="pp", bufs=4))
    lmp = ctx.enter_context(tc.tile_pool(name="lm", bufs=1))
    hop = ctx.enter_context(tc.tile_pool(name="ho", bufs=1))
    stp = ctx.enter_context(tc.tile_pool(name="st", bufs=2))
    scrp = ctx.enter_context(tc.tile_pool(name="scr", bufs=4))

    # ---- constants
    ident = const.tile([128, 128], F16, tag="id", name="ident")
    nc.sync.dma_start(ident[:, :], idn_d[:, :])
    hs_t = const.tile([NMEL, 30], F16, tag="hs", name="hs")
    nc.sync.dma_start(hs_t[:, :], hs_d[:, :])
    hd_t = const.tile([NMEL, 30], F16, tag="hd", name="hd")
    nc.sync.dma_start(hd_t[:, :], hd_d[:, :])
    wt = {}
    WNAMES = (("g0c", g0c_d), ("g1c", g1c_d), ("g0s", g0s_d), ("g1s", g1s_d))
    for nm, d in WNAMES:
        t = const.tile([128, 4, NB], F8, tag=nm, name=nm)
        nc.gpsimd.dma_start(t[:, :, 0:NB // 2], d[:, :, 0:NB // 2])
        wt[nm] = t
    melw = const.tile([128, 16, 64], F8, tag="melw", name="melw")
    nc.gpsimd.dma_start(melw[:, :, :], melw_d[:, :, :])
    for nm, d in WNAMES:
        nc.gpsimd.dma_start(wt[nm][:, :, NB // 2:NB], d[:, :, NB // 2:NB])
    eps_t = const.tile([128, 1], F32, tag="eps", name="eps")
    nc.vector.memset(eps_t[:, :], EPS)

    # ---- phase A: load + cast fp8 + transpose to et[r] = [128, 4, 1024]
    et = {}

    def phase_a_open(r):
        etr = etp.tile([128, 4, NK + 24], F8, tag=f"et{r}", name=f"et{r}")
        nc.vector.memset(etr[:, 3, :], 0.0)   # pad rows 442..511 (0..57 rewritten)
        et[r] = etr

    def phase_a_chunks(r, c_lo, c_hi):
        etr = et[r]
        for ci in range(c_lo, c_hi):
            k0, ksz = ECH[ci]
            e2 = e2p.tile([128, W + 1], F16, tag="e2", name="e2")
            src = bass.AP(xpad, r * (L + 1) + W * k0, [[W, ksz], [1, W + 1]])
            nc.sync.dma_start(e2[0:ksz, :], src)
            ptr4 = ptrp.tile([128, 4, 128], F16, tag="ptr", name="ptr4")
            for rb, (rb0, rbsz) in enumerate(RBL):
                nc.tensor.matmul(ptr4[0:rbsz, rb, 0:ksz], e2[0:ksz, rb0:rb0 + rbsz],
                                 ident[0:ksz, 0:ksz], is_transpose=True,
                                 skip_group_check=True)
            nc.vector.tensor_copy(etr[:, 0:3, k0:k0 + ksz], ptr4[:, 0:3, 0:ksz])
            nc.vector.tensor_copy(etr[0:58, 3, k0:k0 + ksz], ptr4[0:58, 3, 0:ksz])

    # ---- phase B: DFT power -> mel -> log (all fp8 DoubleRow)
    lmt = {}

    def phase_b_fc(r, fci, inject_pe=None, inject_act=None):
        if r not in lmt:
            lmt[r] = lmp.tile([NMEL, T], F16, tag=f"lm{r}", name=f"lm{r}")
        lmr = lmt[r]
        for (f0, fN) in [FCH[fci]]:
            mp = melp.tile([NMEL, 512], F32, tag="mp", name="mp")
            pend = None                     # delayed mel matmul (ACT slack)
            for bc in range(8):
                pq = dftp.tile([128, 2, 512], F32, tag="pq", name="pq")
                for half, w0, w1 in ((0, wt["g0c"], wt["g1c"]),
                                     (1, wt["g0s"], wt["g1s"])):
                    for mi, (wgt, a, u) in enumerate(
                            ((w0, 0, 0), (w0, 0, 1), (w1, 1, 0), (w1, 1, 1))):
                        nc.tensor.matmul(
                            pq[:, half, 0:fN],
                            wgt[:, 2 * u:2 * u + 2, bc * 128:(bc + 1) * 128],
                            et[r][:, 2 * u:2 * u + 2, f0 + a:f0 + a + fN],
                            start=(mi == 0), stop=(mi == 3), perf_mode=DR,
                            skip_group_check=True)
                    if pend is not None:    # one-bc-delayed mel accumulation
                        nc.tensor.matmul(*pend[0], **pend[1])
                        pend = None
                pp = ppp.tile([128, 2, 512], F8, tag="pp", name="pp")
                nc.scalar.activation(pp[:, :, 0:fN], pq[:, :, 0:fN], AF.Square,
                                     scale=XSC)
                if bc == 3 and inject_pe is not None:
                    inject_pe()
                if bc == 5 and inject_act is not None:
                    inject_act()
                pend = ((mp[0:NMEL, 0:fN],
                         melw[:, 2 * bc:2 * bc + 2, 0:NMEL],
                         pp[:, :, 0:fN]),
                        dict(start=(bc == 0), stop=(bc == 7), perf_mode=DR,
                             skip_group_check=True))
            nc.tensor.matmul(*pend[0], **pend[1])
            nc.scalar.activation(lmr[0:NMEL, f0:f0 + fN], mp[0:NMEL, 0:fN],
                                 AF.Ln, bias=eps_t[0:NMEL, :], scale=1.0 / (XSC * XSC))
        return lmr

    # ---- phase C: Haar / delta / stats (fc-sliced, decentered, ACT accum)
    feats = {}
    parts = {}   # r -> (s1p, s2p) [30, 3, 2]

    def c_open(r):
        ca = hop.tile([30, T], F32, tag=f"ca{r}", name=f"ca{r}")
        cd = hop.tile([30, T], F32, tag=f"cd{r}", name=f"cd{r}")
        dl = hop.tile([30, T], F32, tag=f"dl{r}", name=f"dl{r}")
        s1p = stp.tile([30, 3, 2], F32, tag=f"s1p{r}", name=f"s1p{r}")
        s2p = stp.tile([30, 3, 2], F32, tag=f"s2p{r}", name=f"s2p{r}")
        feats[r] = (ca, dl, cd)
        parts[r] = (s1p, s2p)

    def c_slice(r, fci):
        f0, fN = FCH[fci]
        if fci == 0:
            return [(feats[r][0], f0, f0 + fN), (feats[r][1], 0, f0 + fN - 1),
                    (feats[r][2], f0, f0 + fN)]
        return [(feats[r][0], f0, f0 + fN), (feats[r][1], f0 - 1, T),
                (feats[r][2], f0, f0 + fN)]

    def c_haar_mm(r, fci):
        f0, fN = FCH[fci]
        ca, dl, cd = feats[r]
        lmr = lmt[r]
        pca = haarp.tile([30, 512], F32, tag="pca", name="pca")
        nc.tensor.matmul(pca[:, 0:fN], hs_t[:, :], lmr[0:NMEL, f0:f0 + fN],
                         start=True, stop=True, skip_group_check=True)
        nc.vector.tensor_copy(ca[:, f0:f0 + fN], pca[:, 0:fN])
        pcd = haarp.tile([30, 512], F32, tag="pcd", name="pcd")
        nc.tensor.matmul(pcd[:, 0:fN], hd_t[:, :], lmr[0:NMEL, f0:f0 + fN],
                         start=True, stop=True, skip_group_check=True)
        nc.vector.tensor_copy(cd[:, f0:f0 + fN], pcd[:, 0:fN])

    def c_delta_s1(r, fci):
        f0, fN = FCH[fci]
        ca, dl, cd = feats[r]
        s1p, _ = parts[r]
        if fci == 0:
            nc.vector.tensor_sub(dl[:, 0:1], ca[:, 1:2], ca[:, 0:1])
            nc.vector.tensor_sub(dl[:, 1:f0 + fN - 1], ca[:, 2:f0 + fN],
                                 ca[:, 0:f0 + fN - 2])
        else:
            nc.vector.tensor_sub(dl[:, f0 - 1:f0 + fN - 1], ca[:, f0:f0 + fN],
                                 ca[:, f0 - 2:f0 + fN - 2])
            nc.vector.tensor_sub(dl[:, T - 1:T], ca[:, T - 1:T],
                                 ca[:, T - 2:T - 1])
        for si, (feat, c0, c1) in enumerate(c_slice(r, fci)):
            nc.vector.tensor_reduce(s1p[:, si, fci:fci + 1], feat[:, c0:c1],
                                    axis=mybir.AxisListType.X,
                                    op=mybir.AluOpType.add)

    def c_sq(r, fci):
        # in-B slices ride the idle gpsimd queue; post-B slices use ACT
        # (free after phase B) with fused accumulation
        _, s2p = parts[r]
        for si, (feat, c0, c1) in enumerate(c_slice(r, fci)):
            scr = scrp.tile([30, 512], F32, tag="scr", name="scr")
            if (r, fci) != (1, 1):
                nc.gpsimd.tensor_mul(scr[:, 0:c1 - c0], feat[:, c0:c1],
                                     feat[:, c0:c1])
                nc.vector.tensor_reduce(s2p[:, si, fci:fci + 1],
                                        scr[:, 0:c1 - c0],
                                        axis=mybir.AxisListType.X,
                                        op=mybir.AluOpType.add)
            else:
                nc.scalar.activation(scr[:, 0:c1 - c0], feat[:, c0:c1],
                                     AF.Square,
                                     accum_out=s2p[:, si, fci:fci + 1])

    def c_final():
        for r in range(ROWS):
            s1p, s2p = parts[r]
            stats = stp.tile([30, 6], F32, tag=f"st{r}", name=f"st{r}")
            s1 = stp.tile([30, 3], F32, tag="s1", name="s1")
            s2 = stp.tile([30, 3], F32, tag="s2", name="s2")
            nc.vector.tensor_add(s1[:, :], s1p[:, :, 0], s1p[:, :, 1])
            nc.vector.tensor_add(s2[:, :], s2p[:, :, 0], s2p[:, :, 1])
            nc.vector.tensor_scalar_mul(stats[:, 0:3], s1[:, :], 1.0 / (T * SQRT2))
            m2 = stp.tile([30, 3], F32, tag="m2", name="m2")
            nc.vector.tensor_mul(m2[:, :], s1[:, :], s1[:, :])
            nc.vector.tensor_scalar_mul(m2[:, :], m2[:, :], -1.0 / T)
            va = stp.tile([30, 3], F32, tag="va", name="va")
            nc.vector.tensor_add(va[:, :], s2[:, :], m2[:, :])
            nc.scalar.activation(stats[:, 3:6], va[:, :], AF.Sqrt,
                                 scale=1.0 / ((T - 1) * 2.0))
            nc.sync.dma_start(bass.AP(out_d, r * 180, [[1, 180]]), stats[:, :])

    c_open(0)
    c_open(1)
    phase_a_open(0)
    phase_a_chunks(0, 0, 5)
    phase_b_fc(0, 0)
    phase_a_chunks(0, 5, 8)
    phase_b_fc(0, 1, inject_pe=lambda: c_haar_mm(0, 0))
    phase_a_open(1)
    phase_a_chunks(1, 0, 5)
    c_delta_s1(0, 0)
    phase_b_fc(1, 0, inject_pe=lambda: c_haar_mm(0, 1))
    c_sq(0, 0)
    c_delta_s1(0, 1)
    c_sq(0, 1)
    phase_a_chunks(1, 5, 8)
    phase_b_fc(1, 1, inject_pe=lambda: c_haar_mm(1, 0))
    c_delta_s1(1, 0)
    c_sq(1, 0)
    c_haar_mm(1, 1)
    c_delta_s1(1, 1)
    c_sq(1, 1)
    c_final()

_CACHE = {}


def _build():
    if "nc" in _CACHE:
        return _CACHE["nc"]
    nc = bacc.Bacc("TRN2", target_bir_lowering=False, debug=False,
                   enable_asserts=False, num_devices=8)
    xpad = nc.dram_tensor("xpad", [ROWS, L + 1], F16, kind="ExternalInput")
    g0c_d = nc.dram_tensor("g0c", [128, 4, NB], F8, kind="ExternalInput")
    g1c_d = nc.dram_tensor("g1c", [128, 4, NB], F8, kind="ExternalInput")
    g0s_d = nc.dram_tensor("g0s", [128, 4, NB], F8, kind="ExternalInput")
    g1s_d = nc.dram_tensor("g1s", [128, 4, NB], F8, kind="ExternalInput")
    melw_d = nc.dram_tensor("melw", [128, 16, 64], F8, kind="ExternalInput")
    idn_d = nc.dram_tensor("idn", [128, 128], F16, kind="ExternalInput")
    hs_d = nc.dram_tensor("hsum", [NMEL, 30], F16, kind="ExternalInput")
    hd_d = nc.dram_tensor("hdif", [NMEL, 30], F16, kind="ExternalInput")
    out_d = nc.dram_tensor("out", [ROWS, 180], F32, kind="ExternalOutput")
    with tile.TileContext(nc) as tc, ExitStack() as ctx:
        _body(ctx, tc, xpad, g0c_d, g1c_d, g0s_d, g1s_d, melw_d, idn_d,
              hs_d, hd_d, out_d)
    nc.compile()
    _CACHE["nc"] = nc
    return nc


def make_in_maps(waveform: np.ndarray, mel_filters: np.ndarray):
    g0c, g1c, g0s, g1s, melw, idn, hsum, hdif = _host_constants(mel_filters)
    in_maps = []
    for core in range(8):
        rows = waveform[ROWS * core:ROWS * (core + 1)]
        xpad = np.zeros((ROWS, L + 1), np.float16)
        xpad[:, 1:] = rows.astype(np.float16)
        in_maps.append({"xpad": xpad, "g0c": g0c, "g1c": g1c, "g0s": g0s,
                        "g1s": g1s, "melw": melw, "idn": idn,
                        "hsum": hsum, "hdif": hdif})
    return in_maps


def gather_out(results):
    full = np.concatenate([results[c]["out"] for c in range(8)], axis=0)
    return np.ascontiguousarray(
        full.reshape(B, 30, 6).transpose(0, 2, 1).reshape(B, 180)).astype(np.float32)


def run(waveform, mel_filters, trace=False):
    nc = _build()
    in_maps = make_in_maps(np.asarray(waveform, np.float32),
                           np.asarray(mel_filters, np.float32))
    res = run_bass_kernel_spmd(nc, in_maps, core_ids=list(range(8)), trace=trace)
    return gather_out(res.results), res


def kernel(waveform: np.ndarray, mel_filters: np.ndarray) -> np.ndarray:
    out, _ = run(waveform, mel_filters, trace=False)
    return out


# revision 4
# speedup vs baseline: 1.1060x; 1.0035x over previous
"""MFDWC feature extractor, v16: v14 + multi-queue head DMAs.

Differences vs v1:
  - Pre-emphasis is folded into the DFT chunk matrices (contraction runs
    over 442-sample raw chunks; x[-1]=0 pad gives emph[0]=x[0] exactly),
    so phase 1 is just cast fp32->fp8 + PE transpose.
  - DFT and mel matmuls run in fp8e4 with perf_mode=DoubleRow (2 k-subtiles
    per instruction): weights [128, 2, M] / moving [128, 2, N] APs.
  - Power is computed as (X/64)^2 in fp8 (range fits e4m3); the 4096x is
    restored inside the log: Ln(4096*mel' + eps) == Ln(mel + eps) exactly.
  - Phases ordered A(r0) A(r1) B(r0) B(r1) C(r0) C(r1) so the per-row
    stats tails don't head-of-line-block the other row's pipeline.
"""

import math
from contextlib import ExitStack

import numpy as np
import ml_dtypes

import concourse.bass as bass
import concourse.bacc as bacc
import concourse.mybir as mybir
import concourse.tile as tile
from concourse.bass_utils import run_bass_kernel_spmd

F32 = mybir.dt.float32
F16 = mybir.dt.float16
F8 = mybir.dt.float8e4
AF = mybir.ActivationFunctionType
DR = mybir.MatmulPerfMode.DoubleRow
F8NP = ml_dtypes.float8_e4m3

B = 16               # batch
L = 441000           # samples per row
W = 441              # hop; chunks of 442 rows (x[441t-1 .. 441t+440])
NK = 1000            # chunk columns per row
T = 999              # frames per row
NB = 1024            # matmul bins (Nyquist packed into sin col 0)
NMEL = 60
ROWS = 2             # batch rows per core
EPS = 1e-10
SQRT2 = math.sqrt(2.0)
XSC = 1.0 / 64.0     # fold |X|/64 into the squares; 4096x restored in Ln

# chunk column tiling (8 DMA chunks over the 1000 columns)
ECH = [(k * 128, min(128, NK - k * 128)) for k in range(8)]
# transpose row-blocks over the 442 rows per chunk
RBL = [(0, 128), (128, 128), (256, 128), (384, 58)]
# frame chunks (PSUM free-dim <= 512 fp32)
FCH = [(0, 512), (512, 487)]


def _host_constants(mel_filters: np.ndarray):
    j = np.arange(882, dtype=np.float64)
    b = np.arange(NB, dtype=np.float64)
    ham = np.hamming(882)
    ang = 2.0 * np.pi * np.outer(j, b) / 2048.0
    cw = ham[:, None] * np.cos(ang)                               # (882, 1024)
    sw = ham[:, None] * np.sin(ang)
    sw[:, 0] = ham * np.cos(np.pi * j)                            # Nyquist col

    def fold(M):
        # chunk-matrix pair with pre-emphasis folded in (see fold_check.py)
        G0 = np.zeros((512, NB))
        G1 = np.zeros((512, NB))
        G0[1:442, :] = M[0:441, :]
        G0[0:442, :] -= 0.97 * M[0:442, :]
        G1[1:441, :] = M[441:881, :] - 0.97 * M[442:882, :]
        G1[441, :] = M[881, :]
        # [512, 1024] -> [128, 4*1024] (row p holds matrix rows 128g+p at g*1024+b)
        return np.ascontiguousarray(
            G0.reshape(4, 128, NB).transpose(1, 0, 2)
        ).astype(F8NP), np.ascontiguousarray(
            G1.reshape(4, 128, NB).transpose(1, 0, 2)
        ).astype(F8NP)

    g0c, g1c = fold(cw)
    g0s, g1s = fold(sw)

    m = mel_filters.astype(np.float64)                            # (60, 1025)
    mat = m[:, 0:NB]                                              # for |cos| part
    mbt = np.concatenate([m[:, NB:NB + 1], m[:, 1:NB]], axis=1)   # Nyquist in col 0
    # melW[p, 2*bc + {0,1}, m] = {mat,mbt}[m, bc*128 + p], padded m->64
    melw = np.zeros((128, 16, 64), np.float64)
    for bc in range(8):
        melw[:, 2 * bc + 0, 0:NMEL] = mat[:, bc * 128:(bc + 1) * 128].T
        melw[:, 2 * bc + 1, 0:NMEL] = mbt[:, bc * 128:(bc + 1) * 128].T
    melw = np.ascontiguousarray(melw).astype(F8NP)

    idn = np.eye(128, dtype=np.float16)
    hsum = np.zeros((NMEL, 30), np.float16)
    hdif = np.zeros((NMEL, 30), np.float16)
    for i in range(30):
        hsum[2 * i, i] = 1.0
        hsum[2 * i + 1, i] = 1.0
        hdif[2 * i, i] = 1.0
        hdif[2 * i + 1, i] = -1.0
    return g0c, g1c, g0s, g1s, melw, idn, hsum, hdif


def _body(ctx: ExitStack, tc, xpad, g0c_d, g1c_d, g0s_d, g1s_d, melw_d, idn_d,
          hs_d, hd_d, out_d):
    nc = tc.nc

    const = ctx.enter_context(tc.tile_pool(name="const", bufs=1))
    e2p = ctx.enter_context(tc.tile_pool(name="e2", bufs=8))
    etp = ctx.enter_context(tc.tile_pool(name="et", bufs=1))
    ptrp = ctx.enter_context(tc.tile_pool(name="ptr", bufs=1, space="PSUM"))
    dftp = ctx.enter_context(tc.tile_pool(name="dft", bufs=2, space="PSUM"))
    haarp = ctx.enter_context(tc.tile_pool(name="haar", bufs=1, space="PSUM"))
    melp = ctx.enter_context(tc.tile_pool(name="mel", bufs=1, space="PSUM"))
    ppp = ctx.enter_context(tc.tile_pool(name="pp", bufs=4))
    lmp = ctx.enter_context(tc.tile_pool(name="lm", bufs=1))
    hop = ctx.enter_context(tc.tile_pool(name="ho", bufs=1))
    stp = ctx.enter_context(tc.tile_pool(name="st", bufs=2))
    scrp = ctx.enter_context(tc.tile_pool(name="scr", bufs=4))

    # ---- constants
    ident = const.tile([128, 128], F16, tag="id", name="ident")
    nc.sync.dma_start(ident[:, :], idn_d[:, :])
    wt = {}
    WNAMES = (("g0c", g0c_d), ("g1c", g1c_d), ("g0s", g0s_d), ("g1s", g1s_d))
    for qi, (nm, d) in enumerate(WNAMES):
        t = const.tile([128, 4, NB], F8, tag=nm, name=nm)
        q = nc.gpsimd if qi < 2 else nc.scalar
        q.dma_start(t[:, :, 0:NB // 2], d[:, :, 0:NB // 2])
        wt[nm] = t
    melw = const.tile([128, 16, 64], F8, tag="melw", name="melw")
    nc.gpsimd.dma_start(melw[:, :, :], melw_d[:, :, :])
    for qi, (nm, d) in enumerate(WNAMES):
        q = nc.gpsimd if qi < 2 else nc.scalar
        q.dma_start(wt[nm][:, :, NB // 2:NB], d[:, :, NB // 2:NB])
    hs_t = const.tile([NMEL, 30], F16, tag="hs", name="hs")
    nc.scalar.dma_start(hs_t[:, :], hs_d[:, :])
    hd_t = const.tile([NMEL, 30], F16, tag="hd", name="hd")
    nc.scalar.dma_start(hd_t[:, :], hd_d[:, :])
    eps_t = const.tile([128, 1], F32, tag="eps", name="eps")
    nc.vector.memset(eps_t[:, :], EPS)

    # ---- phase A: load + cast fp8 + transpose to et[r] = [128, 4, 1024]
    et = {}

    def phase_a_open(r):
        etr = etp.tile([128, 4, NK + 24], F8, tag=f"et{r}", name=f"et{r}")
        nc.vector.memset(etr[:, 3, :], 0.0)   # pad rows 442..511 (0..57 rewritten)
        et[r] = etr

    def phase_a_chunks(r, c_lo, c_hi):
        etr = et[r]
        for ci in range(c_lo, c_hi):
            k0, ksz = ECH[ci]
            e2 = e2p.tile([128, W + 1], F16, tag="e2", name="e2")
            src = bass.AP(xpad, r * (L + 1) + W * k0, [[W, ksz], [1, W + 1]])
            nc.sync.dma_start(e2[0:ksz, :], src)
            ptr4 = ptrp.tile([128, 4, 128], F16, tag="ptr", name="ptr4")
            for rb, (rb0, rbsz) in enumerate(RBL):
                nc.tensor.matmul(ptr4[0:rbsz, rb, 0:ksz], e2[0:ksz, rb0:rb0 + rbsz],
                                 ident[0:ksz, 0:ksz], is_transpose=True,
                                 skip_group_check=True)
            nc.vector.tensor_copy(etr[:, 0:3, k0:k0 + ksz], ptr4[:, 0:3, 0:ksz])
            nc.vector.tensor_copy(etr[0:58, 3, k0:k0 + ksz], ptr4[0:58, 3, 0:ksz])

    # ---- phase B: DFT power -> mel -> log (all fp8 DoubleRow)
    lmt = {}

    def phase_b_fc(r, fci, inject_pe=None, inject_act=None):
        if r not in lmt:
            lmt[r] = lmp.tile([NMEL, T], F16, tag=f"lm{r}", name=f"lm{r}")
        lmr = lmt[r]
        for (f0, fN) in [FCH[fci]]:
            mp = melp.tile([NMEL, 512], F32, tag="mp", name="mp")
            pend = None                     # delayed mel matmul (ACT slack)
            for bc in range(8):
                pq = dftp.tile([128, 2, 512], F32, tag="pq", name="pq")
                for half, w0, w1 in ((0, wt["g0c"], wt["g1c"]),
                                     (1, wt["g0s"], wt["g1s"])):
                    for mi, (wgt, a, u) in enumerate(
                            ((w0, 0, 0), (w0, 0, 1), (w1, 1, 0), (w1, 1, 1))):
                        nc.tensor.matmul(
                            pq[:, half, 0:fN],
                            wgt[:, 2 * u:2 * u + 2, bc * 128:(bc + 1) * 128],
                            et[r][:, 2 * u:2 * u + 2, f0 + a:f0 + a + fN],
                            start=(mi == 0), stop=(mi == 3), perf_mode=DR,
                            skip_group_check=True)
                    if pend is not None:    # one-bc-delayed mel accumulation
                        nc.tensor.matmul(*pend[0], **pend[1])
                        pend = None
                pp = ppp.tile([128, 2, 512], F8, tag="pp", name="pp")
                nc.scalar.activation(pp[:, :, 0:fN], pq[:, :, 0:fN], AF.Square,
                                     scale=XSC)
                if bc == 3 and inject_pe is not None:
                    inject_pe()
                if bc == 5 and inject_act is not None:
                    inject_act()
                pend = ((mp[0:NMEL, 0:fN],
                         melw[:, 2 * bc:2 * bc + 2, 0:NMEL],
                         pp[:, :, 0:fN]),
                        dict(start=(bc == 0), stop=(bc == 7), perf_mode=DR,
                             skip_group_check=True))
            nc.tensor.matmul(*pend[0], **pend[1])
            nc.scalar.activation(lmr[0:NMEL, f0:f0 + fN], mp[0:NMEL, 0:fN],
                                 AF.Ln, bias=eps_t[0:NMEL, :], scale=1.0 / (XSC * XSC))
        return lmr

    # ---- phase C: Haar / delta / stats (fc-sliced, decentered, ACT accum)
    feats = {}
    parts = {}   # r -> (s1p, s2p) [30, 3, 2]

    def c_open(r):
        ca = hop.tile([30, T], F32, tag=f"ca{r}", name=f"ca{r}")
        cd = hop.tile([30, T], F32, tag=f"cd{r}", name=f"cd{r}")
        dl = hop.tile([30, T], F32, tag=f"dl{r}", name=f"dl{r}")
        s1p = stp.tile([30, 3, 2], F32, tag=f"s1p{r}", name=f"s1p{r}")
        s2p = stp.tile([30, 3, 2], F32, tag=f"s2p{r}", name=f"s2p{r}")
        feats[r] = (ca, dl, cd)
        parts[r] = (s1p, s2p)

    def c_slice(r, fci):
        f0, fN = FCH[fci]
        if fci == 0:
            return [(feats[r][0], f0, f0 + fN), (feats[r][1], 0, f0 + fN - 1),
                    (feats[r][2], f0, f0 + fN)]
        return [(feats[r][0], f0, f0 + fN), (feats[r][1], f0 - 1, T),
                (feats[r][2], f0, f0 + fN)]

    def c_haar_mm(r, fci):
        f0, fN = FCH[fci]
        ca, dl, cd = feats[r]
        lmr = lmt[r]
        pca = haarp.tile([30, 512], F32, tag="pca", name="pca")
        nc.tensor.matmul(pca[:, 0:fN], hs_t[:, :], lmr[0:NMEL, f0:f0 + fN],
                         start=True, stop=True, skip_group_check=True)
        nc.vector.tensor_copy(ca[:, f0:f0 + fN], pca[:, 0:fN])
        pcd = haarp.tile([30, 512], F32, tag="pcd", name="pcd")
        nc.tensor.matmul(pcd[:, 0:fN], hd_t[:, :], lmr[0:NMEL, f0:f0 + fN],
                         start=True, stop=True, skip_group_check=True)
        nc.vector.tensor_copy(cd[:, f0:f0 + fN], pcd[:, 0:fN])

    def c_delta_s1(r, fci):
        f0, fN = FCH[fci]
        ca, dl, cd = feats[r]
        s1p, _ = parts[r]
        if fci == 0:
            nc.vector.tensor_sub(dl[:, 0:1], ca[:, 1:2], ca[:, 0:1])
            nc.vector.tensor_sub(dl[:, 1:f0 + fN - 1], ca[:, 2:f0 + fN],
                                 ca[:, 0:f0 + fN - 2])
        else:
            nc.vector.tensor_sub(dl[:, f0 - 1:f0 + fN - 1], ca[:, f0:f0 + fN],
                                 ca[:, f0 - 2:f0 + fN - 2])
            nc.vector.tensor_sub(dl[:, T - 1:T], ca[:, T - 1:T],
                                 ca[:, T - 2:T - 1])
        for si, (feat, c0, c1) in enumerate(c_slice(r, fci)):
            nc.vector.tensor_reduce(s1p[:, si, fci:fci + 1], feat[:, c0:c1],
                                    axis=mybir.AxisListType.X,
                                    op=mybir.AluOpType.add)

    def c_sq(r, fci):
        # in-B slices ride the idle gpsimd queue; post-B slices use ACT
        # (free after phase B) with fused accumulation
        _, s2p = parts[r]
        for si, (feat, c0, c1) in enumerate(c_slice(r, fci)):
            scr = scrp.tile([30, 512], F32, tag="scr", name="scr")
            if (r, fci) != (1, 1):
                nc.gpsimd.tensor_mul(scr[:, 0:c1 - c0], feat[:, c0:c1],
                                     feat[:, c0:c1])
                nc.vector.tensor_reduce(s2p[:, si, fci:fci + 1],
                                        scr[:, 0:c1 - c0],
                                        axis=mybir.AxisListType.X,
                                        op=mybir.AluOpType.add)
            else:
                nc.scalar.activation(scr[:, 0:c1 - c0], feat[:, c0:c1],
                                     AF.Square,
                                     accum_out=s2p[:, si, fci:fci + 1])

    def c_final():
        for r in range(ROWS):
            s1p, s2p = parts[r]
            stats = stp.tile([30, 6], F32, tag=f"st{r}", name=f"st{r}")
            s1 = stp.tile([30, 3], F32, tag="s1", name="s1")
            s2 = stp.tile([30, 3], F32, tag="s2", name="s2")
            nc.vector.tensor_add(s1[:, :], s1p[:, :, 0], s1p[:, :, 1])
            nc.vector.tensor_add(s2[:, :], s2p[:, :, 0], s2p[:, :, 1])
            nc.vector.tensor_scalar_mul(stats[:, 0:3], s1[:, :], 1.0 / (T * SQRT2))
            m2 = stp.tile([30, 3], F32, tag="m2", name="m2")
            nc.vector.tensor_mul(m2[:, :], s1[:, :], s1[:, :])
            nc.vector.tensor_scalar_mul(m2[:, :], m2[:, :], -1.0 / T)
            va = stp.tile([30, 3], F32, tag="va", name="va")
            nc.vector.tensor_add(va[:, :], s2[:, :], m2[:, :])
            nc.scalar.activation(stats[:, 3:6], va[:, :], AF.Sqrt,
                                 scale=1.0 / ((T - 1) * 2.0))
            nc.sync.dma_start(bass.AP(out_d, r * 180, [[1, 180]]), stats[:, :])

    c_open(0)
    c_open(1)
    phase_a_open(0)
    phase_a_chunks(0, 0, 5)
    phase_b_fc(0, 0)
    phase_a_chunks(0, 5, 8)
    phase_b_fc(0, 1, inject_pe=lambda: c_haar_mm(0, 0))
    phase_a_open(1)
    phase_a_chunks(1, 0, 5)
    c_delta_s1(0, 0)
    phase_b_fc(1, 0, inject_pe=lambda: c_haar_mm(0, 1))
    c_sq(0, 0)
    c_delta_s1(0, 1)
    c_sq(0, 1)
    phase_a_chunks(1, 5, 8)
    phase_b_fc(1, 1, inject_pe=lambda: c_haar_mm(1, 0))
    c_delta_s1(1, 0)
    c_sq(1, 0)
    c_haar_mm(1, 1)
    c_delta_s1(1, 1)
    c_sq(1, 1)
    c_final()

_CACHE = {}


def _build():
    if "nc" in _CACHE:
        return _CACHE["nc"]
    nc = bacc.Bacc("TRN2", target_bir_lowering=False, debug=False,
                   enable_asserts=False, num_devices=8)
    xpad = nc.dram_tensor("xpad", [ROWS, L + 1], F16, kind="ExternalInput")
    g0c_d = nc.dram_tensor("g0c", [128, 4, NB], F8, kind="ExternalInput")
    g1c_d = nc.dram_tensor("g1c", [128, 4, NB], F8, kind="ExternalInput")
    g0s_d = nc.dram_tensor("g0s", [128, 4, NB], F8, kind="ExternalInput")
    g1s_d = nc.dram_tensor("g1s", [128, 4, NB], F8, kind="ExternalInput")
    melw_d = nc.dram_tensor("melw", [128, 16, 64], F8, kind="ExternalInput")
    idn_d = nc.dram_tensor("idn", [128, 128], F16, kind="ExternalInput")
    hs_d = nc.dram_tensor("hsum", [NMEL, 30], F16, kind="ExternalInput")
    hd_d = nc.dram_tensor("hdif", [NMEL, 30], F16, kind="ExternalInput")
    out_d = nc.dram_tensor("out", [ROWS, 180], F32, kind="ExternalOutput")
    with tile.TileContext(nc) as tc, ExitStack() as ctx:
        _body(ctx, tc, xpad, g0c_d, g1c_d, g0s_d, g1s_d, melw_d, idn_d,
              hs_d, hd_d, out_d)
    nc.compile()
    _CACHE["nc"] = nc
    return nc


def make_in_maps(waveform: np.ndarray, mel_filters: np.ndarray):
    g0c, g1c, g0s, g1s, melw, idn, hsum, hdif = _host_constants(mel_filters)
    in_maps = []
    for core in range(8):
        rows = waveform[ROWS * core:ROWS * (core + 1)]
        xpad = np.zeros((ROWS, L + 1), np.float16)
        xpad[:, 1:] = rows.astype(np.float16)
        in_maps.append({"xpad": xpad, "g0c": g0c, "g1c": g1c, "g0s": g0s,
                        "g1s": g1s, "melw": melw, "idn": idn,
                        "hsum": hsum, "hdif": hdif})
    return in_maps


def gather_out(results):
    full = np.concatenate([results[c]["out"] for c in range(8)], axis=0)
    return np.ascontiguousarray(
        full.reshape(B, 30, 6).transpose(0, 2, 1).reshape(B, 180)).astype(np.float32)


def run(waveform, mel_filters, trace=False):
    nc = _build()
    in_maps = make_in_maps(np.asarray(waveform, np.float32),
                           np.asarray(mel_filters, np.float32))
    res = run_bass_kernel_spmd(nc, in_maps, core_ids=list(range(8)), trace=trace)
    return gather_out(res.results), res


def kernel(waveform: np.ndarray, mel_filters: np.ndarray) -> np.ndarray:
    out, _ = run(waveform, mel_filters, trace=False)
    return out


# revision 5
# speedup vs baseline: 1.1185x; 1.0113x over previous
"""MFDWC feature extractor, v18: v16 + double-buffered mel bank.

Differences vs v1:
  - Pre-emphasis is folded into the DFT chunk matrices (contraction runs
    over 442-sample raw chunks; x[-1]=0 pad gives emph[0]=x[0] exactly),
    so phase 1 is just cast fp32->fp8 + PE transpose.
  - DFT and mel matmuls run in fp8e4 with perf_mode=DoubleRow (2 k-subtiles
    per instruction): weights [128, 2, M] / moving [128, 2, N] APs.
  - Power is computed as (X/64)^2 in fp8 (range fits e4m3); the 4096x is
    restored inside the log: Ln(4096*mel' + eps) == Ln(mel + eps) exactly.
  - Phases ordered A(r0) A(r1) B(r0) B(r1) C(r0) C(r1) so the per-row
    stats tails don't head-of-line-block the other row's pipeline.
"""

import math
from contextlib import ExitStack

import numpy as np
import ml_dtypes

import concourse.bass as bass
import concourse.bacc as bacc
import concourse.mybir as mybir
import concourse.tile as tile
from concourse.bass_utils import run_bass_kernel_spmd

F32 = mybir.dt.float32
F16 = mybir.dt.float16
F8 = mybir.dt.float8e4
AF = mybir.ActivationFunctionType
DR = mybir.MatmulPerfMode.DoubleRow
F8NP = ml_dtypes.float8_e4m3

B = 16               # batch
L = 441000           # samples per row
W = 441              # hop; chunks of 442 rows (x[441t-1 .. 441t+440])
NK = 1000            # chunk columns per row
T = 999              # frames per row
NB = 1024            # matmul bins (Nyquist packed into sin col 0)
NMEL = 60
ROWS = 2             # batch rows per core
EPS = 1e-10
SQRT2 = math.sqrt(2.0)
XSC = 1.0 / 64.0     # fold |X|/64 into the squares; 4096x restored in Ln

# chunk column tiling (8 DMA chunks over the 1000 columns)
ECH = [(k * 128, min(128, NK - k * 128)) for k in range(8)]
# transpose row-blocks over the 442 rows per chunk
RBL = [(0, 128), (128, 128), (256, 128), (384, 58)]
# frame chunks (PSUM free-dim <= 512 fp32)
FCH = [(0, 512), (512, 487)]


def _host_constants(mel_filters: np.ndarray):
    j = np.arange(882, dtype=np.float64)
    b = np.arange(NB, dtype=np.float64)
    ham = np.hamming(882)
    ang = 2.0 * np.pi * np.outer(j, b) / 2048.0
    cw = ham[:, None] * np.cos(ang)                               # (882, 1024)
    sw = ham[:, None] * np.sin(ang)
    sw[:, 0] = ham * np.cos(np.pi * j)                            # Nyquist col

    def fold(M):
        # chunk-matrix pair with pre-emphasis folded in (see fold_check.py)
        G0 = np.zeros((512, NB))
        G1 = np.zeros((512, NB))
        G0[1:442, :] = M[0:441, :]
        G0[0:442, :] -= 0.97 * M[0:442, :]
        G1[1:441, :] = M[441:881, :] - 0.97 * M[442:882, :]
        G1[441, :] = M[881, :]
        # [512, 1024] -> [128, 4*1024] (row p holds matrix rows 128g+p at g*1024+b)
        return np.ascontiguousarray(
            G0.reshape(4, 128, NB).transpose(1, 0, 2)
        ).astype(F8NP), np.ascontiguousarray(
            G1.reshape(4, 128, NB).transpose(1, 0, 2)
        ).astype(F8NP)

    g0c, g1c = fold(cw)
    g0s, g1s = fold(sw)

    m = mel_filters.astype(np.float64)                            # (60, 1025)
    mat = m[:, 0:NB]                                              # for |cos| part
    mbt = np.concatenate([m[:, NB:NB + 1], m[:, 1:NB]], axis=1)   # Nyquist in col 0
    # melW[p, 2*bc + {0,1}, m] = {mat,mbt}[m, bc*128 + p], padded m->64
    melw = np.zeros((128, 16, 64), np.float64)
    for bc in range(8):
        melw[:, 2 * bc + 0, 0:NMEL] = mat[:, bc * 128:(bc + 1) * 128].T
        melw[:, 2 * bc + 1, 0:NMEL] = mbt[:, bc * 128:(bc + 1) * 128].T
    melw = np.ascontiguousarray(melw).astype(F8NP)

    idn = np.eye(128, dtype=np.float16)
    hsum = np.zeros((NMEL, 30), np.float16)
    hdif = np.zeros((NMEL, 30), np.float16)
    for i in range(30):
        hsum[2 * i, i] = 1.0
        hsum[2 * i + 1, i] = 1.0
        hdif[2 * i, i] = 1.0
        hdif[2 * i + 1, i] = -1.0
    return g0c, g1c, g0s, g1s, melw, idn, hsum, hdif


def _body(ctx: ExitStack, tc, xpad, g0c_d, g1c_d, g0s_d, g1s_d, melw_d, idn_d,
          hs_d, hd_d, out_d):
    nc = tc.nc

    const = ctx.enter_context(tc.tile_pool(name="const", bufs=1))
    e2p = ctx.enter_context(tc.tile_pool(name="e2", bufs=8))
    etp = ctx.enter_context(tc.tile_pool(name="et", bufs=1))
    ptrp = ctx.enter_context(tc.tile_pool(name="ptr", bufs=1, space="PSUM"))
    dftp = ctx.enter_context(tc.tile_pool(name="dft", bufs=2, space="PSUM"))
    haarp = ctx.enter_context(tc.tile_pool(name="haar", bufs=1, space="PSUM"))
    melp = ctx.enter_context(tc.tile_pool(name="mel", bufs=2, space="PSUM"))
    ppp = ctx.enter_context(tc.tile_pool(name="pp", bufs=6))
    lmp = ctx.enter_context(tc.tile_pool(name="lm", bufs=1))
    hop = ctx.enter_context(tc.tile_pool(name="ho", bufs=1))
    stp = ctx.enter_context(tc.tile_pool(name="st", bufs=2))
    scrp = ctx.enter_context(tc.tile_pool(name="scr", bufs=4))

    # ---- constants
    ident = const.tile([128, 128], F16, tag="id", name="ident")
    nc.sync.dma_start(ident[:, :], idn_d[:, :])
    wt = {}
    WNAMES = (("g0c", g0c_d), ("g1c", g1c_d), ("g0s", g0s_d), ("g1s", g1s_d))
    for qi, (nm, d) in enumerate(WNAMES):
        t = const.tile([128, 4, NB], F8, tag=nm, name=nm)
        q = nc.gpsimd if qi < 2 else nc.scalar
        q.dma_start(t[:, :, 0:NB // 2], d[:, :, 0:NB // 2])
        wt[nm] = t
    melw = const.tile([128, 16, 64], F8, tag="melw", name="melw")
    nc.gpsimd.dma_start(melw[:, :, :], melw_d[:, :, :])
    for qi, (nm, d) in enumerate(WNAMES):
        q = nc.gpsimd if qi < 2 else nc.scalar
        q.dma_start(wt[nm][:, :, NB // 2:NB], d[:, :, NB // 2:NB])
    hs_t = const.tile([NMEL, 30], F16, tag="hs", name="hs")
    nc.scalar.dma_start(hs_t[:, :], hs_d[:, :])
    hd_t = const.tile([NMEL, 30], F16, tag="hd", name="hd")
    nc.scalar.dma_start(hd_t[:, :], hd_d[:, :])
    eps_t = const.tile([128, 1], F32, tag="eps", name="eps")
    nc.vector.memset(eps_t[:, :], EPS)

    # ---- phase A: load + cast fp8 + transpose to et[r] = [128, 4, 1024]
    et = {}

    def phase_a_open(r):
        etr = etp.tile([128, 4, NK + 24], F8, tag=f"et{r}", name=f"et{r}")
        nc.vector.memset(etr[:, 3, :], 0.0)   # pad rows 442..511 (0..57 rewritten)
        et[r] = etr

    def phase_a_chunks(r, c_lo, c_hi):
        etr = et[r]
        for ci in range(c_lo, c_hi):
            k0, ksz = ECH[ci]
            e2 = e2p.tile([128, W + 1], F16, tag="e2", name="e2")
            src = bass.AP(xpad, r * (L + 1) + W * k0, [[W, ksz], [1, W + 1]])
            nc.sync.dma_start(e2[0:ksz, :], src)
            ptr4 = ptrp.tile([128, 4, 128], F16, tag="ptr", name="ptr4")
            for rb, (rb0, rbsz) in enumerate(RBL):
                nc.tensor.matmul(ptr4[0:rbsz, rb, 0:ksz], e2[0:ksz, rb0:rb0 + rbsz],
                                 ident[0:ksz, 0:ksz], is_transpose=True,
                                 skip_group_check=True)
            nc.vector.tensor_copy(etr[:, 0:3, k0:k0 + ksz], ptr4[:, 0:3, 0:ksz])
            nc.vector.tensor_copy(etr[0:58, 3, k0:k0 + ksz], ptr4[0:58, 3, 0:ksz])

    # ---- phase B: DFT power -> mel -> log (all fp8 DoubleRow)
    lmt = {}

    def phase_b_fc(r, fci, inject_pe=None, inject_act=None):
        if r not in lmt:
            lmt[r] = lmp.tile([NMEL, T], F16, tag=f"lm{r}", name=f"lm{r}")
        lmr = lmt[r]
        for (f0, fN) in [FCH[fci]]:
            mp = melp.tile([NMEL, 512], F32, tag="mp", name="mp")
            pend = None                     # delayed mel matmul (ACT slack)
            for bc in range(8):
                pq = dftp.tile([128, 2, 512], F32, tag="pq", name="pq")
                for half, w0, w1 in ((0, wt["g0c"], wt["g1c"]),
                                     (1, wt["g0s"], wt["g1s"])):
                    for mi, (wgt, a, u) in enumerate(
                            ((w0, 0, 0), (w0, 0, 1), (w1, 1, 0), (w1, 1, 1))):
                        nc.tensor.matmul(
                            pq[:, half, 0:fN],
                            wgt[:, 2 * u:2 * u + 2, bc * 128:(bc + 1) * 128],
                            et[r][:, 2 * u:2 * u + 2, f0 + a:f0 + a + fN],
                            start=(mi == 0), stop=(mi == 3), perf_mode=DR,
                            skip_group_check=True)
                    if pend is not None:    # one-bc-delayed mel accumulation
                        nc.tensor.matmul(*pend[0], **pend[1])
                        pend = None
                pp = ppp.tile([128, 2, 512], F8, tag="pp", name="pp")
                nc.scalar.activation(pp[:, :, 0:fN], pq[:, :, 0:fN], AF.Square,
                                     scale=XSC)
                if bc == 3 and inject_pe is not None:
                    inject_pe()
                if bc == 5 and inject_act is not None:
                    inject_act()
                pend = ((mp[0:NMEL, 0:fN],
                         melw[:, 2 * bc:2 * bc + 2, 0:NMEL],
                         pp[:, :, 0:fN]),
                        dict(start=(bc == 0), stop=(bc == 7), perf_mode=DR,
                             skip_group_check=True))
            nc.tensor.matmul(*pend[0], **pend[1])
            nc.scalar.activation(lmr[0:NMEL, f0:f0 + fN], mp[0:NMEL, 0:fN],
                                 AF.Ln, bias=eps_t[0:NMEL, :], scale=1.0 / (XSC * XSC))
        return lmr

    # ---- phase C: Haar / delta / stats (fc-sliced, decentered, ACT accum)
    feats = {}
    parts = {}   # r -> (s1p, s2p) [30, 3, 2]

    def c_open(r):
        ca = hop.tile([30, T], F32, tag=f"ca{r}", name=f"ca{r}")
        cd = hop.tile([30, T], F32, tag=f"cd{r}", name=f"cd{r}")
        dl = hop.tile([30, T], F32, tag=f"dl{r}", name=f"dl{r}")
        s1p = stp.tile([30, 3, 2], F32, tag=f"s1p{r}", name=f"s1p{r}")
        s2p = stp.tile([30, 3, 2], F32, tag=f"s2p{r}", name=f"s2p{r}")
        feats[r] = (ca, dl, cd)
        parts[r] = (s1p, s2p)

    def c_slice(r, fci):
        f0, fN = FCH[fci]
        if fci == 0:
            return [(feats[r][0], f0, f0 + fN), (feats[r][1], 0, f0 + fN - 1),
                    (feats[r][2], f0, f0 + fN)]
        return [(feats[r][0], f0, f0 + fN), (feats[r][1], f0 - 1, T),
                (feats[r][2], f0, f0 + fN)]

    def c_haar_mm(r, fci):
        f0, fN = FCH[fci]
        ca, dl, cd = feats[r]
        lmr = lmt[r]
        pca = haarp.tile([30, 512], F32, tag="ph", name="pca")
        nc.tensor.matmul(pca[:, 0:fN], hs_t[:, :], lmr[0:NMEL, f0:f0 + fN],
                         start=True, stop=True, skip_group_check=True)
        nc.vector.tensor_copy(ca[:, f0:f0 + fN], pca[:, 0:fN])
        pcd = haarp.tile([30, 512], F32, tag="ph", name="pcd")
        nc.tensor.matmul(pcd[:, 0:fN], hd_t[:, :], lmr[0:NMEL, f0:f0 + fN],
                         start=True, stop=True, skip_group_check=True)
        nc.vector.tensor_copy(cd[:, f0:f0 + fN], pcd[:, 0:fN])

    def c_delta_s1(r, fci):
        f0, fN = FCH[fci]
        ca, dl, cd = feats[r]
        s1p, _ = parts[r]
        if fci == 0:
            nc.vector.tensor_sub(dl[:, 0:1], ca[:, 1:2], ca[:, 0:1])
            nc.vector.tensor_sub(dl[:, 1:f0 + fN - 1], ca[:, 2:f0 + fN],
                                 ca[:, 0:f0 + fN - 2])
        else:
            nc.vector.tensor_sub(dl[:, f0 - 1:f0 + fN - 1], ca[:, f0:f0 + fN],
                                 ca[:, f0 - 2:f0 + fN - 2])
            nc.vector.tensor_sub(dl[:, T - 1:T], ca[:, T - 1:T],
                                 ca[:, T - 2:T - 1])
        for si, (feat, c0, c1) in enumerate(c_slice(r, fci)):
            nc.vector.tensor_reduce(s1p[:, si, fci:fci + 1], feat[:, c0:c1],
                                    axis=mybir.AxisListType.X,
                                    op=mybir.AluOpType.add)

    def c_sq(r, fci):
        # in-B slices ride the idle gpsimd queue; post-B slices use ACT
        # (free after phase B) with fused accumulation
        _, s2p = parts[r]
        for si, (feat, c0, c1) in enumerate(c_slice(r, fci)):
            scr = scrp.tile([30, 512], F32, tag="scr", name="scr")
            if (r, fci) != (1, 1):
                nc.gpsimd.tensor_mul(scr[:, 0:c1 - c0], feat[:, c0:c1],
                                     feat[:, c0:c1])
                nc.vector.tensor_reduce(s2p[:, si, fci:fci + 1],
                                        scr[:, 0:c1 - c0],
                                        axis=mybir.AxisListType.X,
                                        op=mybir.AluOpType.add)
            else:
                nc.scalar.activation(scr[:, 0:c1 - c0], feat[:, c0:c1],
                                     AF.Square,
                                     accum_out=s2p[:, si, fci:fci + 1])

    def c_final():
        for r in range(ROWS):
            s1p, s2p = parts[r]
            stats = stp.tile([30, 6], F32, tag=f"st{r}", name=f"st{r}")
            s1 = stp.tile([30, 3], F32, tag="s1", name="s1")
            s2 = stp.tile([30, 3], F32, tag="s2", name="s2")
            nc.vector.tensor_add(s1[:, :], s1p[:, :, 0], s1p[:, :, 1])
            nc.vector.tensor_add(s2[:, :], s2p[:, :, 0], s2p[:, :, 1])
            nc.vector.tensor_scalar_mul(stats[:, 0:3], s1[:, :], 1.0 / (T * SQRT2))
            m2 = stp.tile([30, 3], F32, tag="m2", name="m2")
            nc.vector.tensor_mul(m2[:, :], s1[:, :], s1[:, :])
            nc.vector.tensor_scalar_mul(m2[:, :], m2[:, :], -1.0 / T)
            va = stp.tile([30, 3], F32, tag="va", name="va")
            nc.vector.tensor_add(va[:, :], s2[:, :], m2[:, :])
            nc.scalar.activation(stats[:, 3:6], va[:, :], AF.Sqrt,
                                 scale=1.0 / ((T - 1) * 2.0))
            nc.sync.dma_start(bass.AP(out_d, r * 180, [[1, 180]]), stats[:, :])

    c_open(0)
    c_open(1)
    phase_a_open(0)
    phase_a_chunks(0, 0, 5)
    phase_b_fc(0, 0)
    phase_a_chunks(0, 5, 8)
    phase_b_fc(0, 1, inject_pe=lambda: c_haar_mm(0, 0))
    phase_a_open(1)
    phase_a_chunks(1, 0, 5)
    c_delta_s1(0, 0)
    phase_b_fc(1, 0, inject_pe=lambda: c_haar_mm(0, 1))
    c_sq(0, 0)
    c_delta_s1(0, 1)
    c_sq(0, 1)
    phase_a_chunks(1, 5, 8)
    phase_b_fc(1, 1, inject_pe=lambda: c_haar_mm(1, 0))
    c_delta_s1(1, 0)
    c_sq(1, 0)
    c_haar_mm(1, 1)
    c_delta_s1(1, 1)
    c_sq(1, 1)
    c_final()

_CACHE = {}


def _build():
    if "nc" in _CACHE:
        return _CACHE["nc"]
    nc = bacc.Bacc("TRN2", target_bir_lowering=False, debug=False,
                   enable_asserts=False, num_devices=8)
    xpad = nc.dram_tensor("xpad", [ROWS, L + 1], F16, kind="ExternalInput")
    g0c_d = nc.dram_tensor("g0c", [128, 4, NB], F8, kind="ExternalInput")
    g1c_d = nc.dram_tensor("g1c", [128, 4, NB], F8, kind="ExternalInput")
    g0s_d = nc.dram_tensor("g0s", [128, 4, NB], F8, kind="ExternalInput")
    g1s_d = nc.dram_tensor("g1s", [128, 4, NB], F8, kind="ExternalInput")
    melw_d = nc.dram_tensor("melw", [128, 16, 64], F8, kind="ExternalInput")
    idn_d = nc.dram_tensor("idn", [128, 128], F16, kind="ExternalInput")
    hs_d = nc.dram_tensor("hsum", [NMEL, 30], F16, kind="ExternalInput")
    hd_d = nc.dram_tensor("hdif", [NMEL, 30], F16, kind="ExternalInput")
    out_d = nc.dram_tensor("out", [ROWS, 180], F32, kind="ExternalOutput")
    with tile.TileContext(nc) as tc, ExitStack() as ctx:
        _body(ctx, tc, xpad, g0c_d, g1c_d, g0s_d, g1s_d, melw_d, idn_d,
              hs_d, hd_d, out_d)
    nc.compile()
    _CACHE["nc"] = nc
    return nc


def make_in_maps(waveform: np.ndarray, mel_filters: np.ndarray):
    g0c, g1c, g0s, g1s, melw, idn, hsum, hdif = _host_constants(mel_filters)
    in_maps = []
    for core in range(8):
        rows = waveform[ROWS * core:ROWS * (core + 1)]
        xpad = np.zeros((ROWS, L + 1), np.float16)
        xpad[:, 1:] = rows.astype(np.float16)
        in_maps.append({"xpad": xpad, "g0c": g0c, "g1c": g1c, "g0s": g0s,
                        "g1s": g1s, "melw": melw, "idn": idn,
                        "hsum": hsum, "hdif": hdif})
    return in_maps


def gather_out(results):
    full = np.concatenate([results[c]["out"] for c in range(8)], axis=0)
    return np.ascontiguousarray(
        full.reshape(B, 30, 6).transpose(0, 2, 1).reshape(B, 180)).astype(np.float32)


def run(waveform, mel_filters, trace=False):
    nc = _build()
    in_maps = make_in_maps(np.asarray(waveform, np.float32),
                           np.asarray(mel_filters, np.float32))
    res = run_bass_kernel_spmd(nc, in_maps, core_ids=list(range(8)), trace=trace)
    return gather_out(res.results), res


def kernel(waveform: np.ndarray, mel_filters: np.ndarray) -> np.ndarray:
    out, _ = run(waveform, mel_filters, trace=False)
    return out
